# revision 1
# baseline (speedup 1.0000x reference)
"""AssignAttention (MoE-style routing attention) Trainium2 kernel.

Reference computation (per batch b, head h):
    q = query @ wq.T ; k = kv @ wk.T ; v = kv @ wv.T          (per-head slices)
    logits[n, s] = q_h[n] . k_h[s]
    softmax over n + straight-through one-hot argmax over n
    -> forward output is exactly one_hot(argmax_n logits) applied to v:
       out_h[n] = sum_{s : argmax_n' logits[n', s] = n} v_h[s]
    out = concat_h(out_h) @ wp.T + bp

Key algebraic facts used:
  * softmax is monotonic along the argmax axis, so argmax(softmax) ==
    argmax(logits); the softmax itself cancels in the straight-through
    forward pass (y_hard - y_soft + y_soft == y_hard up to 1-ulp).
  * v-projection commutes with the one-hot aggregation:
       one_hot @ (kv @ wv_h.T) == (one_hot @ kv) @ wv_h.T
    so we aggregate raw kv rows per slot and project afterwards, saving
    the full V projection.
  * logits = kv @ C_h where C_h = wk_h.T @ q_h.T is (C, N) per head.
    The C matrices are tiny (0.8% of total FLOPs) and are computed on
    the host in float64, then split hi/lo for the device.

Sharding: data-parallel over B (32 = 8 cores x 4). No collectives.

Numerics: logits via hi/lo bf16 split (3 bf16 matmuls: hi*hi + hi*lo +
lo*hi) keeps logit error ~3e-5 so the argmax routing matches the fp32
reference almost everywhere. Aggregation + output projections in bf16
(their error does not affect routing). Validated offline: rel_err 4.3e-3
vs the fp32 reference (gate is 2e-2).
"""

import os
import sys
from contextlib import ExitStack

import numpy as np

sys.path.insert(0, "/opt/trn_rl_repo")

import concourse.bass as bass  # noqa: E402
import concourse.tile as tile  # noqa: E402
from concourse import mybir  # noqa: E402
from concourse.bass_utils import run_bass_kernel_spmd  # noqa: E402


def _ensure_walrus_wait_patches():
    """This neuronxcc/walrus build rejects more than one sync-wait command
    per engine instruction ("Too many sync wait commands"). Spread excess
    waits over nofuse NOPs on the same engine (in-order queues make this
    semantically identical). Applied as runtime monkeypatches so kernel.py
    works even on an unpatched concourse checkout; no-ops if tile.py
    already carries the fix."""
    TC = tile.TileContext
    if getattr(TC, "_walrus_wait_patch", False) or hasattr(
        TC, "_hoist_excess_waits"
    ):
        return
    TC._walrus_wait_patch = True

    def _hoist(self, inst):
        if isinstance(
            inst,
            (
                tile.BassTileCriticalSection,
                tile.BassTileRelease,
                tile.BassTileBranchHintPlaceholder,
            ),
        ) or bass.is_branch_inst(inst):
            return
        si = getattr(inst, "sync_info", None)
        if si is None or len(si.on_wait) <= 1:
            return
        eng = getattr(inst, "engine", None)
        if eng is None:
            return
        waits = list(si.on_wait)
        for w in waits[:-1]:
            nop = mybir.InstNoOp(
                name=self.nc.get_next_instruction_name(),
                sync_info=mybir.SyncInfo(on_wait=[w], on_update=[]),
                bass_nofuse=True,
                engine=eng,
            )
            self._commit_instruction(nop, lazy_reg_writes=False)
        inst.sync_info = mybir.SyncInfo(
            on_wait=waits[-1:], on_update=list(si.on_update)
        )

    _orig_commit = TC._commit_and_lower

    def _commit_and_lower(self, inst, original_block, old_bb_map, bb_to_exit_bb):
        _hoist(self, inst)
        return _orig_commit(self, inst, original_block, old_bb_map, bb_to_exit_bb)

    TC._commit_and_lower = _commit_and_lower

    _orig_drain = TC._drain_and_barrier

    def _drain_and_barrier(self, tick_clock, wait_clock):
        ret = None
        try:
            ret = _orig_drain(self, tick_clock, wait_clock)
        finally:
            pass
        return ret

    # the final drain gets its waits inside _orig_drain; split them there by
    # wrapping the sync.drain call is intrusive -- instead post-process is not
    # possible, so replicate the original body with the split applied.
    from concourse.vector_clock import ScopedClock

    def _drain_and_barrier2(self, tick_clock, wait_clock):
        drain_inst = self.nc.sync.drain()
        wait_clock.add_sem_waits(
            drain_inst.ins, ScopedClock({None: tick_clock.global_clock})
        )
        si = drain_inst.ins.sync_info
        if si is not None and len(si.on_wait) > 1:
            extra = list(si.on_wait[1:])
            drain_inst.ins.sync_info = mybir.SyncInfo(
                on_wait=list(si.on_wait[:1]), on_update=list(si.on_update)
            )
            for w in extra:
                f = self.nc.sync.drain(fusable=False)
                f.ins.sync_info = mybir.SyncInfo(on_wait=[w], on_update=[])
        self.nc.all_engine_barrier()
        assert self.sems is not None
        popped = self.nc._tile_sem_poison_stack.pop()
        assert popped is self._sem_poison
        self.nc.clear_and_free_semaphores(list(self.sems.allocated().values()))
        self.nc.all_engine_barrier()

    TC._drain_and_barrier = _drain_and_barrier2


_ensure_walrus_wait_patches()

import ml_dtypes  # noqa: E402

BF16 = ml_dtypes.bfloat16
F32 = np.float32

B, N, S, C, H = 32, 64, 4096, 768, 12
HD = C // H  # 64
NCORES = 8
BL = B // NCORES  # batches per core
DC = C // 128  # 6 d-chunks
SSUB = 128  # keys per logits tile
KT_BLK = 512  # keys per kvT DMA block
AG_BLK = 1024  # keys per aggregation/psum-accumulation block
N_ABLK = S // AG_BLK
N_KBLK = AG_BLK // KT_BLK
N_J = AG_BLK // SSUB  # s_subs per agg block

f32 = mybir.dt.float32
bf16 = mybir.dt.bfloat16

_CACHE = {}
LAST_RESULT = None


def build_nc():
    stage = os.environ.get("KERNEL_STAGE", "full")
    nc = bass.Bass()
    chi_d = nc.declare_dram_parameter("C_hi", [BL, C, C], bf16, isOutput=False)
    clo_d = nc.declare_dram_parameter("C_lo", [BL, C, C], bf16, isOutput=False)
    kvhi_d = nc.declare_dram_parameter("kvT_hi", [BL, C, S], bf16, isOutput=False)
    kvlo_d = nc.declare_dram_parameter("kvT_lo", [BL, C, S], bf16, isOutput=False)
    kvbf_d = nc.declare_dram_parameter("kv_bf", [BL, S, C], bf16, isOutput=False)
    wvT_d = nc.declare_dram_parameter("wvT", [C, C], bf16, isOutput=False)
    wpT_d = nc.declare_dram_parameter("wpT", [C, C], bf16, isOutput=False)
    bp_d = nc.declare_dram_parameter("bp", [1, C], f32, isOutput=False)
    out_d = nc.declare_dram_parameter("out", [BL, N, C], f32, isOutput=True)

    HALVES = ((0, 512), (512, 256))

    with ExitStack() as ctx:
        tc = ctx.enter_context(tile.TileContext(nc))
        wpool = ctx.enter_context(tc.tile_pool(name="w", bufs=1))
        cpool = ctx.enter_context(tc.tile_pool(name="cmat", bufs=2))
        kpool = ctx.enter_context(tc.tile_pool(name="kvT", bufs=2))
        bpool = ctx.enter_context(tc.tile_pool(name="blk", bufs=2))
        spool = ctx.enter_context(tc.tile_pool(name="small", bufs=3))
        apool = ctx.enter_context(tc.tile_pool(name="agg", bufs=2))
        opool = ctx.enter_context(tc.tile_pool(name="outp", bufs=2))
        psL = ctx.enter_context(tc.tile_pool(name="psL", bufs=2, space="PSUM"))
        psA = ctx.enter_context(tc.tile_pool(name="psA", bufs=2, space="PSUM"))

        wpool_tiles = {}

        for b in range(BL):
            C_hi = cpool.tile([128, DC, C], bf16, tag="Chi")
            C_lo = cpool.tile([128, DC, C], bf16, tag="Clo")
            if b == 0:
                # startup-critical: split across DMA queues for parallel arrival
                for dc in range(DC):
                    nc.sync.dma_start(
                        C_hi[:, dc, :], chi_d[b, dc * 128 : (dc + 1) * 128, :]
                    )
                    nc.sync.dma_start(
                        C_lo[:, dc, :], clo_d[b, dc * 128 : (dc + 1) * 128, :]
                    )
            else:
                nc.sync.dma_start(
                    C_hi[:], chi_d[b].rearrange("(c p) e -> p c e", p=128)
                )
                nc.sync.dma_start(
                    C_lo[:], clo_d[b].rearrange("(c p) e -> p c e", p=128)
                )

            agg_t = apool.tile([128, DC, C], f32, tag="agg")
            agg_bf = opool.tile([128, DC, C], bf16, tag="aggbf")

            for ablk in range(N_ABLK):
                a0 = ablk * AG_BLK
                kvbf_t = bpool.tile([128, N_J, C], bf16, tag="kvbf")
                nc.sync.dma_start(
                    kvbf_t[:],
                    kvbf_d[b, a0 : a0 + AG_BLK, :].rearrange("(j p) c -> p j c", p=128),
                )
                oh_t = bpool.tile([128, N_J, C], bf16, tag="oh")

                for kblk in range(N_KBLK):
                    s0 = a0 + kblk * KT_BLK
                    kvhi_t = kpool.tile([128, DC, KT_BLK], bf16, tag="kvhi")
                    kvlo_t = kpool.tile([128, DC, KT_BLK], bf16, tag="kvlo")
                    if b == 0 and ablk == 0 and kblk == 0:
                        for dc in range(DC):
                            nc.sync.dma_start(
                                kvhi_t[:, dc, :],
                                kvhi_d[b, dc * 128 : (dc + 1) * 128, s0 : s0 + KT_BLK],
                            )
                            nc.sync.dma_start(
                                kvlo_t[:, dc, :],
                                kvlo_d[b, dc * 128 : (dc + 1) * 128, s0 : s0 + KT_BLK],
                            )
                    else:
                        nc.sync.dma_start(
                            kvhi_t[:],
                            kvhi_d[b, :, s0 : s0 + KT_BLK].rearrange(
                                "(c p) s -> p c s", p=128
                            ),
                        )
                        nc.sync.dma_start(
                            kvlo_t[:],
                            kvlo_d[b, :, s0 : s0 + KT_BLK].rearrange(
                                "(c p) s -> p c s", p=128
                            ),
                        )

                    for j4 in range(KT_BLK // SSUB):
                        j = kblk * (KT_BLK // SSUB) + j4
                        sl = slice(j4 * SSUB, (j4 + 1) * SSUB)
                        ps_l = psL.tile([128, 768], f32, tag="ps")
                        for dc in range(DC):
                            mms = (
                                (kvhi_t, C_hi),
                                (kvhi_t, C_lo),
                                (kvlo_t, C_hi),
                            )
                            for pi, (kt, ct) in enumerate(mms):
                                for off, w in HALVES:
                                    nc.tensor.matmul(
                                        ps_l[:, off : off + w],
                                        lhsT=kt[:, dc, sl],
                                        rhs=ct[:, dc, off : off + w],
                                        start=(dc == 0 and pi == 0),
                                        stop=(dc == DC - 1 and pi == 2),
                                    )
                        # one-hot of per-column (key) argmax over the 64 slots
                        m_t = spool.tile([128, H, 1], f32, tag="m")
                        nc.vector.tensor_reduce(
                            m_t[:, :, 0],
                            ps_l[:].rearrange("p (h n) -> p h n", h=H),
                            axis=mybir.AxisListType.X,
                            op=mybir.AluOpType.max,
                        )
                        mb_t = spool.tile([128, C], f32, tag="mb")
                        nc.scalar.copy(
                            mb_t[:].rearrange("p (h n) -> p h n", h=H),
                            m_t[:].broadcast_to([128, H, N]),
                        )
                        nc.vector.tensor_tensor(
                            oh_t[:, j, :], ps_l[:], mb_t[:], op=mybir.AluOpType.is_equal
                        )

                if b == 0 and ablk == 0:
                    # epilogue-only loads, issued after the startup-critical
                    # C/kvT DMAs so they don't delay the first matmuls
                    bp_t = wpool.tile([N, C], f32)
                    nc.sync.dma_start(bp_t[:], bp_d[0:1, :].broadcast_to([N, C]))
                    wvT_t = wpool.tile([128, DC, C], bf16, tag="wv")
                    nc.sync.dma_start(
                        wvT_t[:], wvT_d.rearrange("(c p) e -> p c e", p=128)
                    )
                    wpT_t = wpool.tile([128, DC, C], bf16, tag="wp")
                    nc.sync.dma_start(
                        wpT_t[:], wpT_d.rearrange("(c p) e -> p c e", p=128)
                    )
                    wpool_tiles.update(bp=bp_t, wv=wvT_t, wp=wpT_t)
                bp_t = wpool_tiles["bp"]
                wvT_t = wpool_tiles["wv"]
                wpT_t = wpool_tiles["wp"]

                if stage == "logits":
                    continue

                # aggregate: agg.T[d, h*64+n] += sum_s kv[s, d] * oh[s, h*64+n]
                for g in range(DC // 2):
                    pa = [
                        psA.tile([128, 768], f32, tag="pa", name=f"pa{i}")
                        for i in range(2)
                    ]
                    for j in range(N_J):
                        for i in range(2):
                            dc = 2 * g + i
                            for off, w in HALVES:
                                nc.tensor.matmul(
                                    pa[i][:, off : off + w],
                                    lhsT=kvbf_t[:, j, dc * 128 : (dc + 1) * 128],
                                    rhs=oh_t[:, j, off : off + w],
                                    start=(j == 0),
                                    stop=(j == N_J - 1),
                                )
                    for i in range(2):
                        dc = 2 * g + i
                        if ablk == 0:
                            nc.scalar.copy(agg_t[:, dc, :], pa[i][:])
                        elif ablk == N_ABLK - 1:
                            # final flush: emit the bf16 epilogue operand directly
                            nc.vector.tensor_tensor(
                                agg_bf[:, dc, :],
                                agg_t[:, dc, :],
                                pa[i][:],
                                op=mybir.AluOpType.add,
                            )
                        else:
                            nc.vector.tensor_tensor(
                                agg_t[:, dc, :],
                                agg_t[:, dc, :],
                                pa[i][:],
                                op=mybir.AluOpType.add,
                            )

            if stage in ("logits", "agg"):
                nc.sync.dma_start(out_d[b], bp_t[:])
                continue

            # ---- epilogue (bf16): y.T_h = wv_h.T^T @ agg.T_h ; out = y.T^T @ wp.T + bp
            yT_t = opool.tile([128, DC, N], bf16, tag="yT")
            for h in range(H):
                ps_y = psA.tile([128, 768], f32, tag="pa", name="ps_y")
                for dc in range(DC):
                    nc.tensor.matmul(
                        ps_y[0:HD, 0:N],
                        lhsT=wvT_t[:, dc, h * HD : (h + 1) * HD],
                        rhs=agg_bf[:, dc, h * N : (h + 1) * N],
                        start=(dc == 0),
                        stop=(dc == DC - 1),
                    )
                po = (h % 2) * 64
                nc.scalar.copy(yT_t[po : po + 64, h // 2, :], ps_y[0:HD, 0:N])

            ps_o = psL.tile([128, 768], f32, tag="ps")
            for dc in range(DC):
                for off, w in HALVES:
                    nc.tensor.matmul(
                        ps_o[0:N, off : off + w],
                        lhsT=yT_t[:, dc, :],
                        rhs=wpT_t[:, dc, off : off + w],
                        start=(dc == 0),
                        stop=(dc == DC - 1),
                    )
            out_t = opool.tile([N, C], f32, tag="out")
            nc.vector.tensor_tensor(
                out_t[:], ps_o[0:N, :], bp_t[:], op=mybir.AluOpType.add
            )
            nc.sync.dma_start(out_d[b], out_t[:])

    return nc


def _split_hl(x):
    hi = x.astype(BF16)
    lo = (x - hi.astype(F32)).astype(BF16)
    return hi, lo


def _prep_inputs(query, kv, wq, wk, wv, wp, bp):
    """Host-side layout prep + per-core sharding."""
    query = np.ascontiguousarray(query, F32)
    kv = np.ascontiguousarray(kv, F32)
    wvT = np.ascontiguousarray(np.asarray(wv, F32).T).astype(BF16)
    wpT = np.ascontiguousarray(np.asarray(wp, F32).T).astype(BF16)
    bp2 = np.ascontiguousarray(np.asarray(bp, F32).reshape(1, C))

    # C[b, d, h*N+n] = sum_hd wk[h*HD+hd, d] * q[b, n, h*HD+hd] in float64
    q64 = query.astype(np.float64) @ np.asarray(wq, F32).T.astype(np.float64)
    qh = q64.reshape(B, N, H, HD)  # b n h k
    wkr = np.asarray(wk, F32).astype(np.float64).reshape(H, HD, C)  # h k d
    Call = np.einsum("hkd,bnhk->bdhn", wkr, qh, optimize=True).astype(F32)
    Call = Call.reshape(B, C, H * N)

    in_maps = []
    for i in range(NCORES):
        sl = slice(i * BL, (i + 1) * BL)
        kv_i = kv[sl]  # (BL, S, C)
        kvT = np.ascontiguousarray(kv_i.transpose(0, 2, 1))  # (BL, C, S)
        kvT_hi, kvT_lo = _split_hl(kvT)
        C_hi, C_lo = _split_hl(Call[sl])
        in_maps.append(
            {
                "C_hi": C_hi,
                "C_lo": C_lo,
                "kvT_hi": kvT_hi,
                "kvT_lo": kvT_lo,
                "kv_bf": kv_i.astype(BF16),
                "wvT": wvT,
                "wpT": wpT,
                "bp": bp2,
            }
        )
    return in_maps


def kernel(query, kv, wq, wk, wv, wp, bp):
    global LAST_RESULT
    verbose = bool(os.environ.get("KERNEL_VERBOSE"))
    import time as _time

    t0 = _time.time()
    if "nc" not in _CACHE:
        _CACHE["nc"] = build_nc()
    nc = _CACHE["nc"]
    if verbose:
        print(f"[kernel] graph built at {_time.time()-t0:.1f}s", flush=True)
    in_maps = _prep_inputs(query, kv, wq, wk, wv, wp, bp)
    if verbose:
        print(f"[kernel] inputs prepped at {_time.time()-t0:.1f}s", flush=True)
    res = run_bass_kernel_spmd(nc, in_maps, core_ids=list(range(NCORES)))
    if verbose:
        print(f"[kernel] executed at {_time.time()-t0:.1f}s", flush=True)
    LAST_RESULT = res
    out = np.concatenate([res.results[i]["out"] for i in range(NCORES)], axis=0)
    return out.astype(np.float32)


if __name__ == "__main__":
    rng = np.random.default_rng(0)
    inputs = {
        "query": rng.standard_normal((B, N, C)).astype(F32),
        "kv": rng.standard_normal((B, S, C)).astype(F32),
        "wq": (rng.standard_normal((C, C)) * 0.02).astype(F32),
        "wk": (rng.standard_normal((C, C)) * 0.02).astype(F32),
        "wv": (rng.standard_normal((C, C)) * 0.02).astype(F32),
        "wp": (rng.standard_normal((C, C)) * 0.02).astype(F32),
        "bp": np.zeros((C,), F32),
    }
    out = kernel(**inputs)
    print("kernel output:", out.shape, out.dtype)



# revision 3
# speedup vs baseline: 1.6864x; 1.6864x over previous
"""AssignAttention (MoE-style routing attention) Trainium2 kernel.

Reference computation (per batch b, head h):
    q = query @ wq.T ; k = kv @ wk.T ; v = kv @ wv.T          (per-head slices)
    logits[n, s] = q_h[n] . k_h[s]
    softmax over n + straight-through one-hot argmax over n
    -> forward output is exactly one_hot(argmax_n logits) applied to v:
       out_h[n] = sum_{s : argmax_n' logits[n', s] = n} v_h[s]
    out = concat_h(out_h) @ wp.T + bp

Key algebraic facts used:
  * softmax is monotonic along the argmax axis, so argmax(softmax) ==
    argmax(logits); the softmax itself cancels in the straight-through
    forward pass (y_hard - y_soft + y_soft == y_hard up to 1-ulp).
  * v-projection commutes with the one-hot aggregation:
       one_hot @ (kv @ wv_h.T) == (one_hot @ kv) @ wv_h.T
    so we aggregate raw kv rows per slot and project afterwards, saving
    the full V projection.
  * logits = kv @ C_h where C_h = wk_h.T @ q_h.T is (C, N) per head.
    The C matrices are tiny (0.8% of total FLOPs) and are computed on
    the host in float64, then cast to fp16 for the device.

Sharding: data-parallel over B (32 = 8 cores x 4). No collectives.

Numerics: single fp16 matmul for the logits. The PE upconverts fp16 to
FP22 and multiplies exactly (11-bit significands -> 22-bit products,
fp32 accumulation), so logit error is pure operand rounding,
~8.5e-4 abs (logit sigma ~2.4). Validated offline against the fp32
reference with exact flip accounting: 530/1.57M routing flips,
end-to-end rel_err 0.0185 (gate 2e-2). Aggregation + output
projections also fp16 (error contribution ~1e-3 in quadrature).
This replaces the previous bf16 hi/lo 3-matmul logits scheme: 1/3 the
tensor-engine work on the logits pass, which dominated.
"""

import os
import sys
from contextlib import ExitStack

import numpy as np

sys.path.insert(0, "/opt/trn_rl_repo")

import concourse.bass as bass  # noqa: E402
import concourse.tile as tile  # noqa: E402
from concourse import mybir  # noqa: E402
from concourse.bass_utils import run_bass_kernel_spmd  # noqa: E402


def _ensure_walrus_wait_patches():
    """This neuronxcc/walrus build rejects more than one sync-wait command
    per engine instruction ("Too many sync wait commands"). Spread excess
    waits over nofuse NOPs on the same engine (in-order queues make this
    semantically identical). Applied as runtime monkeypatches so kernel.py
    works even on an unpatched concourse checkout; no-ops if tile.py
    already carries the fix."""
    TC = tile.TileContext
    if getattr(TC, "_walrus_wait_patch", False) or hasattr(
        TC, "_hoist_excess_waits"
    ):
        return
    TC._walrus_wait_patch = True

    def _hoist(self, inst):
        if isinstance(
            inst,
            (
                tile.BassTileCriticalSection,
                tile.BassTileRelease,
                tile.BassTileBranchHintPlaceholder,
            ),
        ) or bass.is_branch_inst(inst):
            return
        si = getattr(inst, "sync_info", None)
        if si is None or len(si.on_wait) <= 1:
            return
        eng = getattr(inst, "engine", None)
        if eng is None:
            return
        waits = list(si.on_wait)
        for w in waits[:-1]:
            nop = mybir.InstNoOp(
                name=self.nc.get_next_instruction_name(),
                sync_info=mybir.SyncInfo(on_wait=[w], on_update=[]),
                bass_nofuse=True,
                engine=eng,
            )
            self._commit_instruction(nop, lazy_reg_writes=False)
        inst.sync_info = mybir.SyncInfo(
            on_wait=waits[-1:], on_update=list(si.on_update)
        )

    _orig_commit = TC._commit_and_lower

    def _commit_and_lower(self, inst, original_block, old_bb_map, bb_to_exit_bb):
        _hoist(self, inst)
        return _orig_commit(self, inst, original_block, old_bb_map, bb_to_exit_bb)

    TC._commit_and_lower = _commit_and_lower

    from concourse.vector_clock import ScopedClock

    def _drain_and_barrier2(self, tick_clock, wait_clock):
        drain_inst = self.nc.sync.drain()
        wait_clock.add_sem_waits(
            drain_inst.ins, ScopedClock({None: tick_clock.global_clock})
        )
        si = drain_inst.ins.sync_info
        if si is not None and len(si.on_wait) > 1:
            extra = list(si.on_wait[1:])
            drain_inst.ins.sync_info = mybir.SyncInfo(
                on_wait=list(si.on_wait[:1]), on_update=list(si.on_update)
            )
            for w in extra:
                f = self.nc.sync.drain(fusable=False)
                f.ins.sync_info = mybir.SyncInfo(on_wait=[w], on_update=[])
        self.nc.all_engine_barrier()
        assert self.sems is not None
        popped = self.nc._tile_sem_poison_stack.pop()
        assert popped is self._sem_poison
        self.nc.clear_and_free_semaphores(list(self.sems.allocated().values()))
        self.nc.all_engine_barrier()

    TC._drain_and_barrier = _drain_and_barrier2


_ensure_walrus_wait_patches()

import ml_dtypes  # noqa: E402

FP16 = np.float16
F32 = np.float32

B, N, S, C, H = 32, 64, 4096, 768, 12
HD = C // H  # 64
NCORES = 8
BL = B // NCORES  # batches per core
DC = C // 128  # 6 d-chunks
SSUB = 128  # keys per logits tile
AG_BLK = 1024  # keys per aggregation/psum-accumulation block
N_ABLK = S // AG_BLK
N_J = AG_BLK // SSUB  # s_subs per agg block

f32 = mybir.dt.float32
f16 = mybir.dt.float16

_CACHE = {}
LAST_RESULT = None


def build_nc():
    stage = os.environ.get("KERNEL_STAGE", "full")
    nc = bass.Bass()
    c_d = nc.declare_dram_parameter("C16", [BL, C, C], f16, isOutput=False)
    kvT_d = nc.declare_dram_parameter("kvT16", [BL, C, S], f16, isOutput=False)
    kv_d = nc.declare_dram_parameter("kv16", [BL, S, C], f16, isOutput=False)
    wvT_d = nc.declare_dram_parameter("wvT", [C, C], f16, isOutput=False)
    wpT_d = nc.declare_dram_parameter("wpT", [C, C], f16, isOutput=False)
    bp_d = nc.declare_dram_parameter("bp", [1, C], f32, isOutput=False)
    out_d = nc.declare_dram_parameter("out", [BL, N, C], f32, isOutput=True)

    HALVES = ((0, 512), (512, 256))

    with ExitStack() as ctx:
        tc = ctx.enter_context(tile.TileContext(nc))
        wpool = ctx.enter_context(tc.tile_pool(name="w", bufs=1))
        cpool = ctx.enter_context(tc.tile_pool(name="cmat", bufs=2))
        kpool = ctx.enter_context(tc.tile_pool(name="kvT", bufs=2))
        bpool = ctx.enter_context(tc.tile_pool(name="blk", bufs=2))
        spool = ctx.enter_context(tc.tile_pool(name="small", bufs=3))
        apool = ctx.enter_context(tc.tile_pool(name="agg", bufs=2))
        opool = ctx.enter_context(tc.tile_pool(name="outp", bufs=2))
        psL = ctx.enter_context(tc.tile_pool(name="psL", bufs=2, space="PSUM"))
        psA = ctx.enter_context(tc.tile_pool(name="psA", bufs=2, space="PSUM"))

        wpool_tiles = {}

        for b in range(BL):
            C_t = cpool.tile([128, DC, C], f16, tag="C16")
            if b == 0:
                # startup-critical: split across DMA queues for parallel arrival
                for dc in range(DC):
                    nc.sync.dma_start(
                        C_t[:, dc, :], c_d[b, dc * 128 : (dc + 1) * 128, :]
                    )
            else:
                nc.sync.dma_start(
                    C_t[:], c_d[b].rearrange("(c p) e -> p c e", p=128)
                )

            agg_t = apool.tile([128, DC, C], f32, tag="agg")
            agg16 = opool.tile([128, DC, C], f16, tag="agg16")

            for ablk in range(N_ABLK):
                a0 = ablk * AG_BLK
                kv_t = bpool.tile([128, N_J, C], f16, tag="kv16")
                if b == 0 and ablk == 0:
                    for jj in range(0, N_J, 2):
                        nc.sync.dma_start(
                            kv_t[:, jj : jj + 2, :],
                            kv_d[
                                b, a0 + jj * 128 : a0 + (jj + 2) * 128, :
                            ].rearrange("(j p) c -> p j c", p=128),
                        )
                else:
                    nc.sync.dma_start(
                        kv_t[:],
                        kv_d[b, a0 : a0 + AG_BLK, :].rearrange(
                            "(j p) c -> p j c", p=128
                        ),
                    )
                oh_t = bpool.tile([128, N_J, C], f16, tag="oh")

                kvT_t = kpool.tile([128, DC, AG_BLK], f16, tag="kvT")
                # per-dc slices: contiguous DRAM rows, parallel queues
                for dc in range(DC):
                    nc.sync.dma_start(
                        kvT_t[:, dc, :],
                        kvT_d[b, dc * 128 : (dc + 1) * 128, a0 : a0 + AG_BLK],
                    )

                for j in range(N_J):
                    sl = slice(j * SSUB, (j + 1) * SSUB)
                    ps_l = psL.tile([128, 768], f32, tag="ps")
                    for dc in range(DC):
                        for off, w in HALVES:
                            nc.tensor.matmul(
                                ps_l[:, off : off + w],
                                lhsT=kvT_t[:, dc, sl],
                                rhs=C_t[:, dc, off : off + w],
                                start=(dc == 0),
                                stop=(dc == DC - 1),
                            )
                    # one-hot of per-column (key) argmax over the 64 slots
                    m_t = spool.tile([128, H, 1], f32, tag="m")
                    nc.vector.tensor_reduce(
                        m_t[:, :, 0],
                        ps_l[:].rearrange("p (h n) -> p h n", h=H),
                        axis=mybir.AxisListType.X,
                        op=mybir.AluOpType.max,
                    )
                    mb_t = spool.tile([128, C], f32, tag="mb")
                    nc.scalar.copy(
                        mb_t[:].rearrange("p (h n) -> p h n", h=H),
                        m_t[:].broadcast_to([128, H, N]),
                    )
                    nc.vector.tensor_tensor(
                        oh_t[:, j, :], ps_l[:], mb_t[:], op=mybir.AluOpType.is_equal
                    )

                if b == 0 and ablk == 0:
                    # epilogue-only loads, issued after the startup-critical
                    # C/kvT DMAs so they don't delay the first matmuls
                    bp_t = wpool.tile([N, C], f32)
                    nc.sync.dma_start(bp_t[:], bp_d[0:1, :].broadcast_to([N, C]))
                    wvT_t = wpool.tile([128, DC, C], f16, tag="wv")
                    nc.sync.dma_start(
                        wvT_t[:], wvT_d.rearrange("(c p) e -> p c e", p=128)
                    )
                    wpT_t = wpool.tile([128, DC, C], f16, tag="wp")
                    nc.sync.dma_start(
                        wpT_t[:], wpT_d.rearrange("(c p) e -> p c e", p=128)
                    )
                    wpool_tiles.update(bp=bp_t, wv=wvT_t, wp=wpT_t)
                bp_t = wpool_tiles["bp"]
                wvT_t = wpool_tiles["wv"]
                wpT_t = wpool_tiles["wp"]

                if stage == "logits":
                    continue

                # aggregate: agg.T[d, h*64+n] += sum_s kv[s, d] * oh[s, h*64+n]
                for g in range(DC // 2):
                    pa = [
                        psA.tile([128, 768], f32, tag="pa", name=f"pa{i}")
                        for i in range(2)
                    ]
                    for j in range(N_J):
                        for i in range(2):
                            dc = 2 * g + i
                            for off, w in HALVES:
                                nc.tensor.matmul(
                                    pa[i][:, off : off + w],
                                    lhsT=kv_t[:, j, dc * 128 : (dc + 1) * 128],
                                    rhs=oh_t[:, j, off : off + w],
                                    start=(j == 0),
                                    stop=(j == N_J - 1),
                                )
                    for i in range(2):
                        dc = 2 * g + i
                        if ablk == 0:
                            nc.scalar.copy(agg_t[:, dc, :], pa[i][:])
                        elif ablk == N_ABLK - 1:
                            # final flush: emit the fp16 epilogue operand directly
                            nc.vector.tensor_tensor(
                                agg16[:, dc, :],
                                agg_t[:, dc, :],
                                pa[i][:],
                                op=mybir.AluOpType.add,
                            )
                        else:
                            nc.vector.tensor_tensor(
                                agg_t[:, dc, :],
                                agg_t[:, dc, :],
                                pa[i][:],
                                op=mybir.AluOpType.add,
                            )

            if stage in ("logits", "agg"):
                nc.sync.dma_start(out_d[b], bp_t[:])
                continue

            # ---- epilogue (fp16): y.T_h = wv_h.T^T @ agg.T_h ; out = y.T^T @ wp.T + bp
            yT_t = opool.tile([128, DC, N], f16, tag="yT")
            for h in range(H):
                ps_y = psA.tile([128, 768], f32, tag="pa", name="ps_y")
                for dc in range(DC):
                    nc.tensor.matmul(
                        ps_y[0:HD, 0:N],
                        lhsT=wvT_t[:, dc, h * HD : (h + 1) * HD],
                        rhs=agg16[:, dc, h * N : (h + 1) * N],
                        start=(dc == 0),
                        stop=(dc == DC - 1),
                    )
                po = (h % 2) * 64
                nc.scalar.copy(yT_t[po : po + 64, h // 2, :], ps_y[0:HD, 0:N])

            ps_o = psL.tile([128, 768], f32, tag="ps")
            for dc in range(DC):
                for off, w in HALVES:
                    nc.tensor.matmul(
                        ps_o[0:N, off : off + w],
                        lhsT=yT_t[:, dc, :],
                        rhs=wpT_t[:, dc, off : off + w],
                        start=(dc == 0),
                        stop=(dc == DC - 1),
                    )
            out_t = opool.tile([N, C], f32, tag="out")
            nc.vector.tensor_tensor(
                out_t[:], ps_o[0:N, :], bp_t[:], op=mybir.AluOpType.add
            )
            nc.sync.dma_start(out_d[b], out_t[:])

    return nc


def _prep_inputs(query, kv, wq, wk, wv, wp, bp):
    """Host-side layout prep + per-core sharding."""
    query = np.ascontiguousarray(query, F32)
    kv = np.ascontiguousarray(kv, F32)
    wvT = np.ascontiguousarray(np.asarray(wv, F32).T).astype(FP16)
    wpT = np.ascontiguousarray(np.asarray(wp, F32).T).astype(FP16)
    bp2 = np.ascontiguousarray(np.asarray(bp, F32).reshape(1, C))

    # C[b, d, h*N+n] = sum_hd wk[h*HD+hd, d] * q[b, n, h*HD+hd] in float64
    q64 = query.astype(np.float64) @ np.asarray(wq, F32).T.astype(np.float64)
    qh = q64.reshape(B, N, H, HD)  # b n h k
    wkr = np.asarray(wk, F32).astype(np.float64).reshape(H, HD, C)  # h k d
    Call = np.einsum("hkd,bnhk->bdhn", wkr, qh, optimize=True)
    Call = Call.reshape(B, C, H * N).astype(F32).astype(FP16)

    kv16 = kv.astype(FP16)
    in_maps = []
    for i in range(NCORES):
        sl = slice(i * BL, (i + 1) * BL)
        kvT16 = np.ascontiguousarray(kv16[sl].transpose(0, 2, 1))  # (BL, C, S)
        in_maps.append(
            {
                "C16": Call[sl],
                "kvT16": kvT16,
                "kv16": kv16[sl],
                "wvT": wvT,
                "wpT": wpT,
                "bp": bp2,
            }
        )
    return in_maps


def kernel(query, kv, wq, wk, wv, wp, bp):
    global LAST_RESULT
    verbose = bool(os.environ.get("KERNEL_VERBOSE"))
    import time as _time

    t0 = _time.time()
    if "nc" not in _CACHE:
        _CACHE["nc"] = build_nc()
    nc = _CACHE["nc"]
    if verbose:
        print(f"[kernel] graph built at {_time.time()-t0:.1f}s", flush=True)
    in_maps = _prep_inputs(query, kv, wq, wk, wv, wp, bp)
    if verbose:
        print(f"[kernel] inputs prepped at {_time.time()-t0:.1f}s", flush=True)
    res = run_bass_kernel_spmd(nc, in_maps, core_ids=list(range(NCORES)))
    if verbose:
        print(f"[kernel] executed at {_time.time()-t0:.1f}s", flush=True)
    LAST_RESULT = res
    out = np.concatenate([res.results[i]["out"] for i in range(NCORES)], axis=0)
    return out.astype(np.float32)


if __name__ == "__main__":
    rng = np.random.default_rng(0)
    inputs = {
        "query": rng.standard_normal((B, N, C)).astype(F32),
        "kv": rng.standard_normal((B, S, C)).astype(F32),
        "wq": (rng.standard_normal((C, C)) * 0.02).astype(F32),
        "wk": (rng.standard_normal((C, C)) * 0.02).astype(F32),
        "wv": (rng.standard_normal((C, C)) * 0.02).astype(F32),
        "wp": (rng.standard_normal((C, C)) * 0.02).astype(F32),
        "bp": np.zeros((C,), F32),
    }
    out = kernel(**inputs)
    print("kernel output:", out.shape, out.dtype)


# revision 7
# speedup vs baseline: 1.7385x; 1.0309x over previous
"""AssignAttention (MoE-style routing attention) Trainium2 kernel.

Reference computation (per batch b, head h):
    q = query @ wq.T ; k = kv @ wk.T ; v = kv @ wv.T          (per-head slices)
    logits[n, s] = q_h[n] . k_h[s]
    softmax over n + straight-through one-hot argmax over n
    -> forward output is exactly one_hot(argmax_n logits) applied to v:
       out_h[n] = sum_{s : argmax_n' logits[n', s] = n} v_h[s]
    out = concat_h(out_h) @ wp.T + bp

Key algebraic facts used:
  * softmax is monotonic along the argmax axis, so argmax(softmax) ==
    argmax(logits); the softmax itself cancels in the straight-through
    forward pass (y_hard - y_soft + y_soft == y_hard up to 1-ulp).
  * v-projection commutes with the one-hot aggregation:
       one_hot @ (kv @ wv_h.T) == (one_hot @ kv) @ wv_h.T
    so we aggregate raw kv rows per slot and project afterwards, saving
    the full V projection.
  * logits = kv @ C_h where C_h = wk_h.T @ q_h.T is (C, N) per head.
    The C matrices are tiny (0.8% of total FLOPs) and are computed on
    the host in float64, then cast to fp16 for the device.

Sharding: data-parallel over B (32 = 8 cores x 4). No collectives.

Numerics: single fp16 matmul for the logits. The PE upconverts fp16 to
FP22 and multiplies exactly (11-bit significands -> 22-bit products,
fp32 accumulation), so logit error is pure operand rounding,
~8.5e-4 abs (logit sigma ~2.4). Validated offline against the fp32
reference with exact flip accounting: 530/1.57M routing flips,
end-to-end rel_err 0.0185 (gate 2e-2). Aggregation + output
projections also fp16 (error contribution ~1e-3 in quadrature).
This replaces the previous bf16 hi/lo 3-matmul logits scheme: 1/3 the
tensor-engine work on the logits pass, which dominated.
"""

import os
import sys
from contextlib import ExitStack

import numpy as np

sys.path.insert(0, "/opt/trn_rl_repo")

try:  # noqa: SIM105 — missing on some images; bass_utils needs it when tracing
    import antenv.axon_hooks  # noqa: F401, E402
except ImportError:
    import types

    _m = types.ModuleType("antenv.axon_hooks")
    _m._hook = None
    _m.set_axon_ntff_profile_hook = lambda h: setattr(_m, "_hook", h)
    _m.get_axon_ntff_profile_hook = lambda: _m._hook
    sys.modules["antenv.axon_hooks"] = _m

import concourse.bass as bass  # noqa: E402
import concourse.tile as tile  # noqa: E402
from concourse import mybir  # noqa: E402
from concourse.bass_utils import run_bass_kernel_spmd  # noqa: E402


def _ensure_walrus_wait_patches():
    """This neuronxcc/walrus build rejects more than one sync-wait command
    per engine instruction ("Too many sync wait commands"). Spread excess
    waits over nofuse NOPs on the same engine (in-order queues make this
    semantically identical). Applied as runtime monkeypatches so kernel.py
    works even on an unpatched concourse checkout; no-ops if tile.py
    already carries the fix."""
    TC = tile.TileContext
    if getattr(TC, "_walrus_wait_patch", False) or hasattr(
        TC, "_hoist_excess_waits"
    ):
        return
    TC._walrus_wait_patch = True

    def _hoist(self, inst):
        if isinstance(
            inst,
            (
                tile.BassTileCriticalSection,
                tile.BassTileRelease,
                tile.BassTileBranchHintPlaceholder,
            ),
        ) or bass.is_branch_inst(inst):
            return
        si = getattr(inst, "sync_info", None)
        if si is None or len(si.on_wait) <= 1:
            return
        eng = getattr(inst, "engine", None)
        if eng is None:
            return
        waits = list(si.on_wait)
        for w in waits[:-1]:
            nop = mybir.InstNoOp(
                name=self.nc.get_next_instruction_name(),
                sync_info=mybir.SyncInfo(on_wait=[w], on_update=[]),
                bass_nofuse=True,
                engine=eng,
            )
            self._commit_instruction(nop, lazy_reg_writes=False)
        inst.sync_info = mybir.SyncInfo(
            on_wait=waits[-1:], on_update=list(si.on_update)
        )

    _orig_commit = TC._commit_and_lower

    def _commit_and_lower(self, inst, original_block, old_bb_map, bb_to_exit_bb):
        _hoist(self, inst)
        return _orig_commit(self, inst, original_block, old_bb_map, bb_to_exit_bb)

    TC._commit_and_lower = _commit_and_lower

    from concourse.vector_clock import ScopedClock

    def _drain_and_barrier2(self, tick_clock, wait_clock):
        drain_inst = self.nc.sync.drain()
        wait_clock.add_sem_waits(
            drain_inst.ins, ScopedClock({None: tick_clock.global_clock})
        )
        si = drain_inst.ins.sync_info
        if si is not None and len(si.on_wait) > 1:
            extra = list(si.on_wait[1:])
            drain_inst.ins.sync_info = mybir.SyncInfo(
                on_wait=list(si.on_wait[:1]), on_update=list(si.on_update)
            )
            for w in extra:
                f = self.nc.sync.drain(fusable=False)
                f.ins.sync_info = mybir.SyncInfo(on_wait=[w], on_update=[])
        self.nc.all_engine_barrier()
        assert self.sems is not None
        popped = self.nc._tile_sem_poison_stack.pop()
        assert popped is self._sem_poison
        self.nc.clear_and_free_semaphores(list(self.sems.allocated().values()))
        self.nc.all_engine_barrier()

    TC._drain_and_barrier = _drain_and_barrier2


_ensure_walrus_wait_patches()

import ml_dtypes  # noqa: E402

FP16 = np.float16
F32 = np.float32

B, N, S, C, H = 32, 64, 4096, 768, 12
HD = C // H  # 64
NCORES = 8
BL = B // NCORES  # batches per core
DC = C // 128  # 6 d-chunks
SSUB = 128  # keys per logits tile
AG_BLK = 1024  # keys per aggregation/psum-accumulation block
N_ABLK = S // AG_BLK
N_J = AG_BLK // SSUB  # s_subs per agg block

f32 = mybir.dt.float32
f16 = mybir.dt.float16

_CACHE = {}
LAST_RESULT = None


def build_nc():
    nc = bass.Bass()
    c_d = nc.declare_dram_parameter("C16", [BL, C, C], f16, isOutput=False)
    kvT_d = nc.declare_dram_parameter("kvT16", [BL, C, S], f16, isOutput=False)
    kv_d = nc.declare_dram_parameter("kv16", [BL, S, C], f16, isOutput=False)
    wvT_d = nc.declare_dram_parameter("wvT", [C, C], f16, isOutput=False)
    wpT_d = nc.declare_dram_parameter("wpT", [C, C], f16, isOutput=False)
    bp_d = nc.declare_dram_parameter("bp", [1, C], f32, isOutput=False)
    out_d = nc.declare_dram_parameter("out", [BL, N, C], f32, isOutput=True)

    HALVES = ((0, 512), (512, 256))

    with ExitStack() as ctx:
        tc = ctx.enter_context(tile.TileContext(nc))
        wpool = ctx.enter_context(tc.tile_pool(name="w", bufs=1))
        cpool = ctx.enter_context(tc.tile_pool(name="cmat", bufs=2))
        kpool = ctx.enter_context(tc.tile_pool(name="kvT", bufs=2))
        bpool = ctx.enter_context(tc.tile_pool(name="blk", bufs=2))
        spool = ctx.enter_context(tc.tile_pool(name="small", bufs=3))
        apool = ctx.enter_context(tc.tile_pool(name="agg", bufs=2))
        opool = ctx.enter_context(tc.tile_pool(name="outp", bufs=2))
        psL = ctx.enter_context(tc.tile_pool(name="psL", bufs=2, space="PSUM"))
        psA = ctx.enter_context(tc.tile_pool(name="psA", bufs=2, space="PSUM"))

        wpool_tiles = {}
        batch_tiles = {}

        def emit_agg(blk):
            """PE aggregation + flush for a finished logits block.

            agg.T[d, h*64+n] += sum_s kv[s, d] * oh[s, h*64+n]
            """
            b, ablk, kv_t, oh_js = blk
            agg_t, agg16 = batch_tiles[b]
            for g in range(DC // 2):
                pa = [
                    psA.tile([128, 768], f32, tag="pa", name=f"pa{i}")
                    for i in range(2)
                ]
                for j in range(N_J):
                    for i in range(2):
                        dc = 2 * g + i
                        for off, w in HALVES:
                            nc.tensor.matmul(
                                pa[i][:, off : off + w],
                                lhsT=kv_t[:, j, dc * 128 : (dc + 1) * 128],
                                rhs=oh_js[j][:, off : off + w],
                                start=(j == 0),
                                stop=(j == N_J - 1),
                            )
                for i in range(2):
                    dc = 2 * g + i
                    if ablk == 0:
                        nc.scalar.copy(agg_t[:, dc, :], pa[i][:])
                    elif ablk == N_ABLK - 1:
                        # final flush: emit the fp16 epilogue operand directly
                        nc.vector.tensor_tensor(
                            agg16[:, dc, :],
                            agg_t[:, dc, :],
                            pa[i][:],
                            op=mybir.AluOpType.add,
                        )
                    else:
                        nc.vector.tensor_tensor(
                            agg_t[:, dc, :],
                            agg_t[:, dc, :],
                            pa[i][:],
                            op=mybir.AluOpType.add,
                        )

        def emit_epilogue(b):
            """y.T_h = wv_h.T^T @ agg.T_h ; out = y.T^T @ wp.T + bp (fp16)."""
            _, agg16 = batch_tiles.pop(b)
            bp_t = wpool_tiles["bp"]
            wvT_t = wpool_tiles["wv"]
            wpT_t = wpool_tiles["wp"]
            yT_t = opool.tile([128, DC, N], f16, tag="yT")
            for h in range(H):
                ps_y = psA.tile([128, 768], f32, tag="pa", name="ps_y")
                for dc in range(DC):
                    nc.tensor.matmul(
                        ps_y[0:HD, 0:N],
                        lhsT=wvT_t[:, dc, h * HD : (h + 1) * HD],
                        rhs=agg16[:, dc, h * N : (h + 1) * N],
                        start=(dc == 0),
                        stop=(dc == DC - 1),
                    )
                po = (h % 2) * 64
                nc.scalar.copy(yT_t[po : po + 64, h // 2, :], ps_y[0:HD, 0:N])

            ps_o = psL.tile([128, 768], f32, tag="ps")
            for dc in range(DC):
                for off, w in HALVES:
                    nc.tensor.matmul(
                        ps_o[0:N, off : off + w],
                        lhsT=yT_t[:, dc, :],
                        rhs=wpT_t[:, dc, off : off + w],
                        start=(dc == 0),
                        stop=(dc == DC - 1),
                    )
            out_t = opool.tile([N, C], f32, tag="out")
            nc.vector.tensor_tensor(
                out_t[:], ps_o[0:N, :], bp_t[:], op=mybir.AluOpType.add
            )
            nc.sync.dma_start(out_d[b], out_t[:])

        prev_blk = None
        for b in range(BL):
            C_t = cpool.tile([128, DC, C], f16, tag="C16")
            if b == 0:
                # startup-critical: split across DMA queues for parallel arrival
                for dc in range(DC):
                    nc.sync.dma_start(
                        C_t[:, dc, :], c_d[b, dc * 128 : (dc + 1) * 128, :]
                    )
            else:
                nc.sync.dma_start(
                    C_t[:], c_d[b].rearrange("(c p) e -> p c e", p=128)
                )

            agg_t = apool.tile([128, DC, C], f32, tag="agg")
            agg16 = opool.tile([128, DC, C], f16, tag="agg16")
            batch_tiles[b] = (agg_t, agg16)

            for ablk in range(N_ABLK):
                a0 = ablk * AG_BLK
                kvT_t = kpool.tile([128, DC, AG_BLK], f16, tag="kvT")
                # per-dc slices: contiguous DRAM rows, parallel queues.
                # At startup the first logits sub-block's keys land first.
                kvT_splits = ((0, SSUB), (SSUB, AG_BLK)) if (
                    b == 0 and ablk == 0
                ) else ((0, AG_BLK),)
                for s0, s1 in kvT_splits:
                    for dc in range(DC):
                        nc.sync.dma_start(
                            kvT_t[:, dc, s0:s1],
                            kvT_d[
                                b, dc * 128 : (dc + 1) * 128, a0 + s0 : a0 + s1
                            ],
                        )
                kv_t = bpool.tile([128, N_J, C], f16, tag="kv16")
                nc.sync.dma_start(
                    kv_t[:],
                    kv_d[b, a0 : a0 + AG_BLK, :].rearrange(
                        "(j p) c -> p j c", p=128
                    ),
                )
                oh_js = [
                    bpool.tile([128, C], f16, tag=f"oh{j}", name=f"oh{j}")
                    for j in range(N_J)
                ]

                if b == 0 and ablk == 0:
                    # epilogue-only loads, issued after the startup-critical
                    # C/kvT DMAs so they don't delay the first matmuls
                    bp_t = wpool.tile([N, C], f32)
                    nc.sync.dma_start(bp_t[:], bp_d[0:1, :].broadcast_to([N, C]))
                    wvT_t = wpool.tile([128, DC, C], f16, tag="wv")
                    nc.sync.dma_start(
                        wvT_t[:], wvT_d.rearrange("(c p) e -> p c e", p=128)
                    )
                    wpT_t = wpool.tile([128, DC, C], f16, tag="wp")
                    nc.sync.dma_start(
                        wpT_t[:], wpT_d.rearrange("(c p) e -> p c e", p=128)
                    )
                    wpool_tiles.update(bp=bp_t, wv=wvT_t, wp=wpT_t)

                # aggregation of the PREVIOUS block goes first: its one-hots
                # are complete, so the PE never waits on the DVE chain; this
                # block's chain overlaps the agg + next logits matmuls.
                if prev_blk is not None:
                    emit_agg(prev_blk)

                for j in range(N_J):
                    sl = slice(j * SSUB, (j + 1) * SSUB)
                    ps_l = psL.tile([128, 768], f32, tag="ps")
                    for dc in range(DC):
                        for off, w in HALVES:
                            nc.tensor.matmul(
                                ps_l[:, off : off + w],
                                lhsT=kvT_t[:, dc, sl],
                                rhs=C_t[:, dc, off : off + w],
                                start=(dc == 0),
                                stop=(dc == DC - 1),
                            )
                    # one-hot of per-column (key) argmax over the 64 slots
                    m_t = spool.tile([128, H, 1], f32, tag="m")
                    nc.vector.tensor_reduce(
                        m_t[:, :, 0],
                        ps_l[:].rearrange("p (h n) -> p h n", h=H),
                        axis=mybir.AxisListType.X,
                        op=mybir.AluOpType.max,
                    )
                    mb_t = spool.tile([128, C], f32, tag="mb")
                    nc.scalar.copy(
                        mb_t[:].rearrange("p (h n) -> p h n", h=H),
                        m_t[:].broadcast_to([128, H, N]),
                    )
                    nc.vector.tensor_tensor(
                        oh_js[j][:], ps_l[:], mb_t[:], op=mybir.AluOpType.is_equal
                    )

                prev_blk = (b, ablk, kv_t, oh_js)

                # previous batch's epilogue: issued after this batch's first
                # logits block so the agg16 flush latency hides under it
                if b > 0 and ablk == 0:
                    emit_epilogue(b - 1)

        emit_agg(prev_blk)
        emit_epilogue(BL - 1)

    return nc


def _prep_inputs(query, kv, wq, wk, wv, wp, bp):
    """Host-side layout prep + per-core sharding."""
    query = np.ascontiguousarray(query, F32)
    kv = np.ascontiguousarray(kv, F32)
    wvT = np.ascontiguousarray(np.asarray(wv, F32).T).astype(FP16)
    wpT = np.ascontiguousarray(np.asarray(wp, F32).T).astype(FP16)
    bp2 = np.ascontiguousarray(np.asarray(bp, F32).reshape(1, C))

    # C[b, d, h*N+n] = sum_hd wk[h*HD+hd, d] * q[b, n, h*HD+hd] in float64
    q64 = query.astype(np.float64) @ np.asarray(wq, F32).T.astype(np.float64)
    qh = q64.reshape(B, N, H, HD)  # b n h k
    wkr = np.asarray(wk, F32).astype(np.float64).reshape(H, HD, C)  # h k d
    Call = np.einsum("hkd,bnhk->bdhn", wkr, qh, optimize=True)
    Call = Call.reshape(B, C, H * N).astype(F32).astype(FP16)

    kv16 = kv.astype(FP16)
    in_maps = []
    for i in range(NCORES):
        sl = slice(i * BL, (i + 1) * BL)
        kvT16 = np.ascontiguousarray(kv16[sl].transpose(0, 2, 1))  # (BL, C, S)
        in_maps.append(
            {
                "C16": Call[sl],
                "kvT16": kvT16,
                "kv16": kv16[sl],
                "wvT": wvT,
                "wpT": wpT,
                "bp": bp2,
            }
        )
    return in_maps


def kernel(query, kv, wq, wk, wv, wp, bp):
    global LAST_RESULT
    verbose = bool(os.environ.get("KERNEL_VERBOSE"))
    import time as _time

    t0 = _time.time()
    if "nc" not in _CACHE:
        _CACHE["nc"] = build_nc()
    nc = _CACHE["nc"]
    if verbose:
        print(f"[kernel] graph built at {_time.time()-t0:.1f}s", flush=True)
    in_maps = _prep_inputs(query, kv, wq, wk, wv, wp, bp)
    if verbose:
        print(f"[kernel] inputs prepped at {_time.time()-t0:.1f}s", flush=True)
    res = run_bass_kernel_spmd(nc, in_maps, core_ids=list(range(NCORES)))
    if verbose:
        print(f"[kernel] executed at {_time.time()-t0:.1f}s", flush=True)
    LAST_RESULT = res
    out = np.concatenate([res.results[i]["out"] for i in range(NCORES)], axis=0)
    return out.astype(np.float32)


if __name__ == "__main__":
    rng = np.random.default_rng(0)
    inputs = {
        "query": rng.standard_normal((B, N, C)).astype(F32),
        "kv": rng.standard_normal((B, S, C)).astype(F32),
        "wq": (rng.standard_normal((C, C)) * 0.02).astype(F32),
        "wk": (rng.standard_normal((C, C)) * 0.02).astype(F32),
        "wv": (rng.standard_normal((C, C)) * 0.02).astype(F32),
        "wp": (rng.standard_normal((C, C)) * 0.02).astype(F32),
        "bp": np.zeros((C,), F32),
    }
    out = kernel(**inputs)
    print("kernel output:", out.shape, out.dtype)
